# revision 8
# baseline (speedup 1.0000x reference)
"""Trainium2 Bass kernel: causal multi-head self-attention (pipelined, bf16).

Problem: B=2, T=4096, C=768, H=12, D=64, causal softmax(QK^T/sqrt(D))V + out proj.

Sharding (8 cores): core c handles batch b=c//4 and 3 heads g=c%4 (rows
192*g:192*(g+1) of wq/wk/wv, same columns of wo). Each core computes its
heads' full attention and a partial out-projection (T, C) for its batch;
the host sums the 4 partials per batch and transposes back to (B, T, C).

Design (HW exec ~319us vs 417us for the phase-separated baseline):
  - Single pipelined loop: projections for chunk ch+1 and out-projection
    for qb-1 are emitted as filler rounds between attention groups, so
    ScalarE exp starts ~10us in and PE bubbles are filled. Fillers are
    drained at each window start (Tile deps are program-order: a consumer
    emitted before its producer would read stale data).
  - Everything the PE streams is bf16: fp32/f32r moving operands run at
    half rate (~601ns vs ~379ns for an N=512 matmul). Inputs are cast to
    bf16 host-side; ctx and the partial output are bf16 too.
  - K^T/Q^T stored with BOTH 64-partition halves holding the same head
    data; score matmuls contract K=128 over the duplicated halves (= 2x
    scores, compensated in the exp scale). K=64 row-packed pairs measure
    no faster (partial-contraction MMs stream at 1.2 GHz), and the dup
    layout needs no zero-fill and keeps HAM at full clock.
  - Diagonal 512x512 block tiled triangularly at 128 granularity:
    scores/exp/ctx only cover the causal part; causal masks shrink to
    N=128 matmuls (identity.T @ trimask pre-accumulated into PSUM).
  - PSUM: sp tag 2 bufs x 3 banks (rotation = double buffering),
    ctx 1 bank, aux (proj + outproj rounds) 1 bank = 8 banks.
  - ctx accumulator is evacuated on ScalarE (not the busy in-order Vector
    queue) so the single ctx bank recycles without stalling the PE queue;
    epilogue out-projection pipelines through the freed sp banks.
  - V carries a ones column per head, so the softmax denominator falls
    out of the ctx matmul; normalization runs off the critical path.
"""

import os
import sys
import types

import numpy as np

if "/opt/trn_rl_repo" not in sys.path:
    sys.path.insert(0, "/opt/trn_rl_repo")

import concourse.bass as bass  # noqa: E402
import concourse.mybir as mybir  # noqa: E402
from concourse import bacc, tile  # noqa: E402
from concourse.bass_utils import run_bass_kernel_spmd  # noqa: E402

F32 = mybir.dt.float32
F32R = mybir.dt.float32r
BF16 = mybir.dt.bfloat16
EXP = mybir.ActivationFunctionType.Exp

B, T, C, H, D = 2, 4096, 768, 12, 64
HPD = 3          # heads per device
DH = HPD * D     # 192 local head channels
NCORES = 8
QB = 512         # query block (PSUM bank)
LT = 128         # key(l)-tile size
GRP = 3          # l-tiles per exp group (3 PSUM banks)


def build_kernel(t=T, trace_sim=False, paired=False):
    n_lt = t // LT
    n_qb = t // QB
    nct = C // 128            # 6
    escale = 0.125 if paired else 0.0625
    half0, half1 = slice(0, 64), slice(64, 128)

    nc = bacc.Bacc("TRN2", target_bir_lowering=False, debug=False,
                   num_devices=NCORES)
    xT_d = nc.dram_tensor("xT", [C, t], BF16, kind="ExternalInput")
    wqT_d = nc.dram_tensor("wqT", [C, DH], BF16, kind="ExternalInput")
    wkT_d = nc.dram_tensor("wkT", [C, DH], BF16, kind="ExternalInput")
    wvT_d = nc.dram_tensor("wvT", [C, DH], BF16, kind="ExternalInput")
    woT_d = nc.dram_tensor("woT", [256, C], BF16, kind="ExternalInput")
    outT_d = nc.dram_tensor("outT", [C, t], BF16, kind="ExternalOutput")

    with tile.TileContext(nc, trace_sim=trace_sim) as tc:
        with (
            tc.tile_pool(name="const", bufs=1) as const,
            tc.tile_pool(name="xs", bufs=2) as xs,
            tc.tile_pool(name="epool", bufs=3) as epool,
            tc.tile_pool(name="small", bufs=3) as small,
            tc.tile_pool(name="pp", bufs=1, space="PSUM") as pp,
        ):
            # ---- weights -------------------------------------------------
            wq_r = wqT_d.ap().rearrange("(ct p) d -> p ct d", p=128)
            wk_r = wkT_d.ap().rearrange("(ct p) d -> p ct d", p=128)
            wq01_s = const.tile([128, nct, 128], BF16)
            wk01_s = const.tile([128, nct, 128], BF16)
            wqk2_s = const.tile([128, nct, 128], BF16)
            wv_s = const.tile([128, nct, DH], BF16)

            def load_weights():
                nc.sync.dma_start(wk01_s[:], wk_r[:, :, 0:128])
                nc.sync.dma_start(wqk2_s[:, :, 0:64], wq_r[:, :, 128:DH])
                nc.sync.dma_start(wqk2_s[:, :, 64:128], wk_r[:, :, 128:DH])
                nc.sync.dma_start(wv_s[:], wvT_d.ap().rearrange(
                    "(ct p) d -> p ct d", p=128))
            woT_a = const.tile([128, C], BF16)
            woT_b = const.tile([128, C], BF16)   # rows 64:128 are host zeros

            # ---- constants -----------------------------------------------
            # strict upper-left causal mask for 128x128 diagonal sub-tiles:
            # trimask[p, f] = -1e30 where p > f (key index above query)
            trimask = const.tile([128, 128], F32)
            nc.gpsimd.memset(trimask[:], 0.0)
            nc.gpsimd.affine_select(
                out=trimask[:], in_=trimask[:],
                compare_op=mybir.AluOpType.is_ge, fill=-1.0e30,
                base=0, channel_multiplier=-1, pattern=[[1, 128]])
            trimaskb = const.tile([128, 128], BF16)
            nc.vector.tensor_copy(trimaskb[:], trimask[:])

            identf = const.tile([128, 128], F32)
            nc.gpsimd.memset(identf[:], 1.0)
            nc.gpsimd.affine_select(
                out=identf[:], in_=identf[:],
                compare_op=mybir.AluOpType.is_equal, fill=0.0,
                base=0, channel_multiplier=1, pattern=[[-1, 128]])
            identb = const.tile([128, 128], BF16)
            nc.vector.tensor_copy(identb[:], identf[:])

            ones1 = const.tile([128, 1], F32)
            nc.vector.memset(ones1[:], 1.0)
            onesrow = const.tile([1, 64], F32)
            nc.vector.memset(onesrow[:], 1.0)
            zero1 = const.tile([128, 1], F32)
            nc.vector.memset(zero1[:], 0.0)

            # ---- persistent activations ----------------------------------
            KT = [const.tile([128, t], BF16, tag=f"kt{h}", name=f"kt{h}")
                  for h in range(HPD)]
            QT = [const.tile([128, t], BF16, tag=f"qt{h}", name=f"qt{h}")
                  for h in range(HPD)]
            Vone = const.tile([128, n_lt, HPD * 65], BF16)
            ctxT01 = const.tile([128, t], BF16)
            ctxT2 = const.tile([128, t], BF16)   # rows 64:128 stay zero
            nc.vector.tensor_copy(ctxT2[:], zero1[:].to_broadcast((128, t)))
            nc.vector.tensor_copy(
                Vone[:].rearrange("p a b -> p (a b)"),
                ones1[:].to_broadcast((128, n_lt * HPD * 65)))

            # ---- filler rounds (projections / out-projection) ------------
            xT_r = xT_d.ap().rearrange("(ct p) t -> p ct t", p=128)
            xc_tiles = {}

            def round_xc(ch):
                def f():
                    xc = xs.tile([128, nct, QB], BF16, tag="xc", name="xc")
                    xc_tiles[ch] = xc
                    cs = slice(ch * QB, (ch + 1) * QB)
                    for ct in range(nct):
                        nc.sync.dma_start(xc[:, ct, :], xT_r[:, ct, cs])
                return f

            def round_qk(ch, which, tag="aux", bufs=1):
                # which: 0 = q heads 0,1; 1 = k heads 0,1; 2 = q2|k2
                def f():
                    cs = slice(ch * QB, (ch + 1) * QB)
                    xc = xc_tiles[ch]
                    w = (wq01_s, wk01_s, wqk2_s)[which]
                    pa = pp.tile([128, QB], F32, tag=tag, bufs=bufs, name="pa")
                    for ct in range(nct):
                        nc.tensor.matmul(pa[:], w[:, ct, :], xc[:, ct, :],
                                         start=(ct == 0), stop=(ct == nct - 1))
                    if which == 0:
                        lo, hi = QT[0], QT[1]
                    elif which == 1:
                        lo, hi = KT[0], KT[1]
                    else:
                        lo, hi = QT[2], KT[2]
                    nc.vector.tensor_copy(lo[0:64, cs], pa[0:64, :])
                    nc.vector.tensor_copy(hi[64:128, cs], pa[64:128, :])
                    # duplicate into the other 64-partition half (DMA is free)
                    nc.sync.dma_start(lo[64:128, cs], lo[0:64, cs])
                    nc.sync.dma_start(hi[0:64, cs], hi[64:128, cs])
                return f

            def round_pv(ch, tpair, tag="aux", bufs=1):
                def f():
                    xc = xc_tiles[ch]
                    pa = pp.tile([128, QB], F32, tag=tag, bufs=bufs, name="pa")
                    ts0, ts1 = 2 * tpair, 2 * tpair + 1
                    for k, ts in enumerate((ts0, ts1)):
                        for ct in range(nct):
                            nc.tensor.matmul(
                                pa[:, k * DH:(k + 1) * DH],
                                xc[:, ct, ts * 128:(ts + 1) * 128],
                                wv_s[:, ct, :],
                                start=(k == 0 and ct == 0),
                                stop=(k == 1 and ct == nct - 1))
                    for k, ts in enumerate((ts0, ts1)):
                        tt = ch * (QB // 128) + ts
                        nc.vector.tensor_copy(
                            Vone[:, tt, 0:HPD * 65].rearrange(
                                "p (h x) -> p h x", x=65)[:, :, 0:64],
                            pa[:, k * DH:(k + 1) * DH].rearrange(
                                "p (h x) -> p h x", x=64))
                return f

            def round_outproj(qb, oc, tag="aux", bufs=1):
                def f():
                    qs = slice(qb * QB, (qb + 1) * QB)
                    ocs = slice(oc * 128, (oc + 1) * 128)
                    po = pp.tile([128, QB], F32, tag=tag, bufs=bufs, name="po")
                    nc.tensor.matmul(po[:], woT_a[:, ocs], ctxT01[:, qs],
                                     start=True, stop=False)
                    nc.tensor.matmul(po[:], woT_b[:, ocs], ctxT2[:, qs],
                                     start=False, stop=True)
                    ot = small.tile([128, QB], BF16, tag="ot", name="ot")
                    nc.vector.tensor_copy(ot[:], po[:])
                    nc.sync.dma_start(outT_d.ap()[ocs, qs], ot[:])
                return f

            pending = []

            def flush(n=None):
                k = len(pending) if n is None else min(n, len(pending))
                for _ in range(k):
                    pending.pop(0)()

            # ---- attention helpers ---------------------------------------
            def hs_of(lt):
                if not paired:
                    return slice(0, 128)
                return half0 if lt % 2 == 0 else half1

            # ---- prologue: chunk 0 (and xc prefetch for chunk 1) ---------
            nc.sync.dma_start(wq01_s[:], wq_r[:, :, 0:128])
            round_xc(0)()
            load_weights()
            if n_qb > 1:
                round_xc(1)()
            for which in range(3):
                round_qk(0, which, tag="sp", bufs=2)()
            round_pv(0, 0, tag="sp", bufs=2)()
            round_pv(0, 1, tag="sp", bufs=2)()

            # woT is first needed by outproj in window 1 (~40us in) —
            # keep its DMAs off the startup descriptor queue
            nc.sync.dma_start(woT_a[:], woT_d.ap()[0:128, :])
            nc.sync.dma_start(woT_b[:], woT_d.ap()[128:256, :])

            # ---- main pipelined loop -------------------------------------
            for qb in range(n_qb):
                qs = slice(qb * QB, (qb + 1) * QB)
                # drain last window's leftovers NOW: this window's attention
                # consumes chunk qb's projections, and Tile dependencies are
                # program-order — a consumer emitted before its producer
                # would read stale data.
                flush()
                if qb + 2 < n_qb:
                    pending.append(round_xc(qb + 2))
                if qb + 1 < n_qb:
                    for which in range(3):
                        pending.append(round_qk(qb + 1, which))
                    pending.append(round_pv(qb + 1, 0))
                    pending.append(round_pv(qb + 1, 1))
                if qb >= 1:
                    for oc in range(nct):
                        pending.append(round_outproj(qb - 1, oc))

                F = 4 * qb                      # full (off-diagonal) l-tiles
                for h in range(HPD):
                    ctxp = pp.tile([65, QB], F32, tag="ctx", name="ctxp")
                    vh = slice(h * 65, h * 65 + 65)
                    started = [False]

                    def emit_ctx(tiles, et, widths=None):
                        for i, lt in enumerate(tiles):
                            if widths is None:
                                src = et[:, i * QB:(i + 1) * QB]
                                dst = ctxp[:, :]
                            else:
                                off, qoff, n, last = widths[i]
                                src = et[:, off:off + n]
                                dst = ctxp[:, qoff:qoff + n]
                            nc.tensor.matmul(
                                dst, Vone[:, lt, vh], src,
                                start=(not started[0]),
                                stop=(widths is not None and widths[i][3]))
                            started[0] = True

                    sizes = [GRP] * (F // GRP) + ([F % GRP] if F % GRP else [])
                    prev = None
                    lt0 = 0
                    for sz in sizes:
                        tiles = list(range(lt0, lt0 + sz))
                        lt0 += sz
                        sp = pp.tile([128, GRP * QB], F32, tag="sp", bufs=2,
                                     name="sp")
                        for i, lt in enumerate(tiles):
                            nc.tensor.matmul(
                                sp[:, i * QB:(i + 1) * QB],
                                KT[h][hs_of(lt), lt * LT:(lt + 1) * LT],
                                QT[h][hs_of(lt), qs],
                                start=True, stop=True)
                        et = epool.tile([128, GRP * QB], BF16, tag="et",
                                        name="et")
                        nc.scalar.activation(et[:, :sz * QB], sp[:, :sz * QB],
                                             EXP, scale=escale)
                        if prev is not None:
                            emit_ctx(*prev)
                            flush(2 if qb <= 2 else 1)
                        prev = (tiles, et)

                    # -- diagonal block: triangular 128-granular tiling ----
                    # psum layout (cols): j0 [0:512], j1 [512:896],
                    # j3 [896:1024], j2 [1024:1280]; j covers q-cols
                    # [j*128:512] of this q-block.
                    dl = 4 * qb
                    spd = pp.tile([128, GRP * QB], F32, tag="sp", bufs=2,
                                  name="spd")

                    def sdiag(j, off, start, stop):
                        n = (4 - j) * 128
                        lt = dl + j
                        nc.tensor.matmul(
                            spd[:, off:off + n],
                            KT[h][hs_of(lt), lt * LT:(lt + 1) * LT],
                            QT[h][hs_of(lt),
                                  qb * QB + j * 128:(qb + 1) * QB],
                            start=start, stop=stop)

                    sdiag(0, 0, True, False)       # bank 0
                    sdiag(1, 512, True, False)     # bank 1 (with j3)
                    sdiag(2, 1024, True, False)    # bank 2
                    sdiag(3, 896, False, False)    # bank 1
                    # causal masks: one N=128 matmul per diagonal sub-tile
                    for off, stop in ((0, True), (512, False), (896, True),
                                      (1024, True)):
                        nc.tensor.matmul(spd[:, off:off + 128], identb[:],
                                         trimaskb[:], start=False, stop=stop)
                    etd = epool.tile([128, GRP * QB], BF16, tag="et",
                                     name="etd")
                    nc.scalar.activation(etd[:, :1280], spd[:, :1280],
                                         EXP, scale=escale)
                    if prev is not None:
                        emit_ctx(*prev)
                        flush(2 if qb <= 2 else 1)
                        prev = None
                    emit_ctx([dl + 0, dl + 1, dl + 3, dl + 2], etd,
                             widths=[(0, 0, 512, False),
                                     (512, 128, 384, False),
                                     (896, 384, 128, False),
                                     (1024, 256, 256, True)])
                    flush(1)

                    # -- normalize (off the critical path) -----------------
                    # denominator row straight from PSUM, before the stg
                    # evacuation: the reciprocal chain starts ~700ns earlier
                    dn = small.tile([1, QB], F32, tag="dn", name="dn")
                    nc.scalar.copy(dn[:], ctxp[64:65, :])
                    stg = small.tile([65, QB], F32, tag="stg", name="stg")
                    # evacuate ctxp on ScalarE: its queue drains right after
                    # this head's exp, so the ctx bank recycles without
                    # waiting on the (busy, in-order) Vector queue
                    nc.scalar.copy(stg[:], ctxp[:])
                    rec = small.tile([1, QB], F32, tag="rec", name="rec")
                    nc.vector.reciprocal_approx_fast(rec[:], dn[:])
                    if qb == n_qb - 1 and h == HPD - 1:
                        # tail: broadcast 1/denom with a K=1 PE matmul — the
                        # PE touch splits the normalize-chain idle below the
                        # HAM re-throttle window, so the epilogue outproj
                        # runs at full clock
                        rbp = pp.tile([64, QB], F32, tag="aux", name="rbp")
                        nc.tensor.matmul(rbp[:], onesrow[:], rec[:],
                                         start=True, stop=True)
                        rb = rbp   # normalize mul reads PSUM directly
                    else:
                        rb = small.tile([64, QB], F32, tag="rb", name="rb")
                        nc.gpsimd.partition_broadcast(rb[:], rec[:])
                    if h == 1:
                        st2 = small.tile([64, QB], BF16, tag="st2", name="st2")
                        nc.vector.tensor_mul(st2[:], stg[0:64, :], rb[:])
                        nc.sync.dma_start(ctxT01[64:128, qs], st2[:])
                    else:
                        dst = ctxT01 if h == 0 else ctxT2
                        nc.vector.tensor_mul(dst[0:64, qs], stg[0:64, :],
                                             rb[:])

            # ---- epilogue ------------------------------------------------
            flush()
            for oc in range(nct):
                round_outproj(n_qb - 1, oc, tag="sp", bufs=2)()

    nc.compile()
    return nc


_NC_CACHE = {}
LAST_EXEC_NS = None
LAST_RES = None


def _get_nc():
    if "full" not in _NC_CACHE:
        _NC_CACHE["full"] = build_kernel()
    return _NC_CACHE["full"]


def _install_ntff_shim():
    """Make run_bass_kernel_spmd(trace=True) work under axon in this image."""
    import antenv
    if "antenv.axon_hooks" in sys.modules:
        return
    mod = types.ModuleType("antenv.axon_hooks")
    mod._hook = None
    mod.set_axon_ntff_profile_hook = lambda h: setattr(mod, "_hook", h)
    mod.get_axon_ntff_profile_hook = lambda: mod._hook
    sys.modules["antenv.axon_hooks"] = mod
    antenv.axon_hooks = mod
    try:
        from trn_agent_boot.trn_boot import _ntff_profile_via_ctypes
        mod.set_axon_ntff_profile_hook(
            _ntff_profile_via_ctypes("/opt/axon/libaxon_pjrt.so"))
    except Exception:
        pass


def make_in_maps(x, wq, wk, wv, wo):
    import ml_dtypes
    bf = ml_dtypes.bfloat16
    x = np.asarray(x, dtype=np.float32).astype(bf)
    wq = np.asarray(wq, dtype=np.float32).astype(bf)
    wk = np.asarray(wk, dtype=np.float32).astype(bf)
    wv = np.asarray(wv, dtype=np.float32).astype(bf)
    wo = np.asarray(wo, dtype=np.float32).astype(bf)
    in_maps = []
    for c in range(NCORES):
        b, g = c // (NCORES // B), c % (NCORES // B)
        rs, re = g * DH, (g + 1) * DH
        woT = np.zeros((256, C), dtype=bf)
        woT[:DH] = wo[:, rs:re].T
        in_maps.append({
            "xT": np.ascontiguousarray(x[b].T),
            "wqT": np.ascontiguousarray(wq[rs:re].T),
            "wkT": np.ascontiguousarray(wk[rs:re].T),
            "wvT": np.ascontiguousarray(wv[rs:re].T),
            "woT": woT,
        })
    return in_maps


def kernel(x, wq, wk, wv, wo):
    global LAST_EXEC_NS, LAST_RES
    in_maps = make_in_maps(x, wq, wk, wv, wo)
    nc = _get_nc()
    trace = bool(int(os.environ.get("KERNEL_TRACE", "0")))
    if trace:
        try:
            _install_ntff_shim()
        except Exception:
            trace = False
    try:
        res = run_bass_kernel_spmd(nc, in_maps, core_ids=list(range(NCORES)),
                                   trace=trace)
    except Exception:
        if not trace:
            raise
        res = run_bass_kernel_spmd(nc, in_maps, core_ids=list(range(NCORES)),
                                   trace=False)
    LAST_EXEC_NS = res.exec_time_ns
    LAST_RES = res
    outT = [res.results[c]["outT"] for c in range(NCORES)]
    halves = []
    for b in range(B):
        acc = outT[4 * b].astype(np.float64)
        for c in range(4 * b + 1, 4 * b + 4):
            acc = acc + outT[c]
        halves.append(acc.T)
    return np.stack(halves).astype(np.float32)
